# revision 1
# baseline (speedup 1.0000x reference)
"""Trainium2 Bass kernel for CustomMultiHeadAttention (single-query pooled attention).

Reference computation (B=32, S=1024, D=256, H=8):
    keys   = (x @ Wk + bk).reshape(B,S,H,D)
    values = (x @ Wv + bv).reshape(B,S,H,D)
    scores = einsum('bshd,hd->bsh', keys, query)
    attn   = softmax(scores, axis=1)           # over S
    pooled = einsum('bsh,bshd->bhd', attn, values).reshape(B, H*D)
    out    = pooled @ Wo + bo

Algebraic restructure (exact in real arithmetic):
    q_proj[e,h] = sum_d Wk[e, h*D+d] * query[h,d]        # [256, 8]
    scores[b,s,h] = x[b,s,:] @ q_proj[:,h]  (+ const(h) from bk -> cancels in softmax)
    attnu = exp(scores - 64)                             # const shift; softmax invariant
    ctx[b,h,e]  = sum_s attnu[b,s,h] * x[b,s,e];  Z[b,h] = sum_s attnu[b,s,h]
    pooled[b,h,:] = (ctx[b,h,:]/Z[b,h]) @ Wv_h + bv_h    # sum_s attn = 1
    out = pooled @ Wo + (bv @ Wo + bo)

This removes both [B*S,256]x[256,2048] projections; the kernel is memory-bound.
Z is obtained free as an extra all-ones column appended to x in the ctx matmul.
Scores use exact fp32 matmuls (cheap: N=8); the post-softmax path uses float32r.
Sharding: data-parallel over batch, 4 batches per core on 8 cores.

Layout note: PE matmul operands/outputs need base partition in {0,32,64}, so
local batches 0..2 sit at partition offsets 0/32/64 and batch 3 uses a second
free-dim slab at offset 0 (only relevant for the tiny [8 x *] ctx tiles).
"""

import sys

sys.path.insert(0, "/opt/trn_rl_repo")

import numpy as np

import concourse.bass as bass
import concourse.mybir as mybir
import concourse.tile as tile
from concourse import bacc
from concourse.bass_utils import run_bass_kernel_spmd
from concourse.masks import make_identity

F32 = mybir.dt.float32
F32R = mybir.dt.float32r

B, S, D, H = 32, 1024, 256, 8
NCORES = 8
BL = B // NCORES      # local batches per core = 4
ST = S // 128         # s-tiles per batch = 8
KD = 2                # 256 = 2 k-tiles of 128 over the D (input dim) axis
KHD = (H * D) // 128  # 16 k-tiles over the H*D axis
SHIFT = 64.0          # constant score shift before exp (softmax-invariant)

def build_program():
    nc = bacc.Bacc("TRN2", target_bir_lowering=False, debug=False)

    xn_d = nc.dram_tensor("xn", [BL, S, D + 2], F32R, kind="ExternalInput")
    wk_d = nc.dram_tensor("wk", [D, H * D], F32, kind="ExternalInput")
    wv_d = nc.dram_tensor("wv", [D, H * D], F32R, kind="ExternalInput")
    wo_d = nc.dram_tensor("wo", [H * D, D], F32R, kind="ExternalInput")
    q_d = nc.dram_tensor("q", [H, D], F32, kind="ExternalInput")
    bv_d = nc.dram_tensor("bv", [H * D], F32, kind="ExternalInput")
    bo_d = nc.dram_tensor("bo", [D], F32R, kind="ExternalInput")
    on_d = nc.dram_tensor("on", [1, BL], F32R, kind="ExternalInput")
    out_d = nc.dram_tensor("out", [BL, D], F32, kind="ExternalOutput")

    with tile.TileContext(nc) as tc:
        with (
            tc.tile_pool(name="big", bufs=1) as big,
            tc.tile_pool(name="sm", bufs=1) as sm,
            tc.tile_pool(name="ps", bufs=1, space=bass.MemorySpace.PSUM) as ps,
            tc.tile_pool(name="pst", bufs=2, space=bass.MemorySpace.PSUM) as pst,
        ):
            # ---- SBUF allocations -------------------------------------
            xn_sb = big.tile([128, BL, ST, D + 2], F32R)  # x natural + 2 ones cols
            xt_sb = big.tile([128, KD, BL, S], F32)       # x transposed: p=e%128
            wk_sb = big.tile([128, KD, H * D], F32)
            wv_sb = big.tile([128, KD, H * D], F32R)
            wo_sb = big.tile([128, KHD, D], F32R)
            qrep = big.tile([128, H * D], F32)            # query replicated
            qsmall = sm.tile([1, H * D], F32)
            tmp = big.tile([128, KD, H * D], F32)         # wk * qrep scratch

            qp = sm.tile([128, KD, H], F32)               # q_proj [e, h]
            attn_sb = sm.tile([128, BL, ST, H], F32R)     # exp(scores-SHIFT) [s, h]
            recip = sm.tile([H, BL, 1], F32)              # 1/Z per (h, b)
            ctx_sb = sm.tile([H, BL, D], F32)             # [h, b, e]
            ctxT_sb = sm.tile([128, KD, BL, H], F32R)     # [e%128, eh, b, h]
            pooledT_sb = sm.tile([128, KHD, BL], F32R)    # [(hd)%128, ktile, b]
            bvn_sb = sm.tile([KHD, 128], F32)             # bv natural [k, p]
            bvT_sb = sm.tile([128, KHD], F32R)
            bo_sb = sm.tile([1, D], F32R)
            bias_sb = sm.tile([1, D], F32R)               # bv @ Wo + bo
            ones_sb = sm.tile([1, BL], F32R)
            ident = sm.tile([16, 16], F32)
            ident128 = sm.tile([128, 128], F32)
            negs = sm.tile([128, 1], F32)                 # -SHIFT bias for exp
            out_sb = sm.tile([BL, D], F32)

            # ---- DMA loads -------------------------------------------
            nc.sync.dma_start(
                qsmall[:], q_d[:].rearrange("h d -> () (h d)")
            )
            nc.gpsimd.partition_broadcast(qrep[:], qsmall[:])
            nc.sync.dma_start(
                wk_sb[:], wk_d[:].rearrange("(k p) f -> p k f", p=128)
            )
            for b in range(BL):
                nc.sync.dma_start(
                    xn_sb[:, b, :, :],
                    xn_d[b].rearrange("(t p) e -> p t e", p=128),
                )
            nc.sync.dma_start(
                wv_sb[:], wv_d[:].rearrange("(k p) f -> p k f", p=128)
            )
            for kh in range(2):
                nc.sync.dma_start(
                    wo_sb[:, kh * 8:(kh + 1) * 8, :],
                    wo_d[kh * 1024:(kh + 1) * 1024, :]
                    .rearrange("(k p) n -> p k n", p=128),
                )
            nc.sync.dma_start(bvn_sb[:], bv_d[:].rearrange("(k p) -> k p", p=128))
            nc.sync.dma_start(bo_sb[:], bo_d[:].rearrange("d -> () d"))
            nc.sync.dma_start(ones_sb[:], on_d[:])

            make_identity(nc, ident[:])
            make_identity(nc, ident128[:])
            nc.vector.memset(negs[:], -SHIFT)

            # ---- q_proj[e,h] = sum_d Wk[e, h*D+d] * query[h,d] --------
            nc.vector.tensor_mul(
                tmp[:],
                wk_sb[:],
                qrep[:].rearrange("p f -> p () f").broadcast_to([128, KD, H * D]),
            )
            nc.vector.reduce_sum(
                qp[:],
                tmp[:].rearrange("p k (h d) -> p k h d", d=D),
                axis=mybir.AxisListType.X,
            )

            # ---- transpose x on chip: xt[e, s] per (b, eh) (PE, fp32) -
            # 4 transposes share one PSUM bank -> one batched DVE copy
            for b in range(BL):
                for tp2 in range(ST // 2):       # pairs of s-tiles
                    xtp = pst.tile([128, 2, 2, 128], F32, tag="xtp")
                    for toff in range(2):
                        t = tp2 * 2 + toff
                        for eh in range(KD):
                            nc.tensor.transpose(
                                xtp[:, toff, eh, :],
                                xn_sb[:, b, t, eh * 128:(eh + 1) * 128].bitcast(F32),
                                ident128[:],
                            )
                    # dest [p, eh, s(2x128)]; src permuted [p, eh, toff, 128]
                    nc.vector.tensor_copy(
                        xt_sb[:, :, b, tp2 * 256:(tp2 + 1) * 256]
                        .rearrange("p k (o s) -> p k o s", o=2),
                        xtp[:].rearrange("p o k s -> p k o s"),
                    )

            # ---- scores[s, h] per (b, s-tile) = xt_tile.T @ q_proj ----
            # out[s, h] = sum_e xt[e, s] * qp[e, h]; exact fp32 (N=8 so cheap)
            scores_ps = ps.tile([128, BL, ST, H], F32, tag="scores")
            for b in range(BL):
                for t in range(ST):
                    for k in range(KD):
                        nc.tensor.matmul(
                            scores_ps[:, b, t, :],
                            xt_sb[:, k, b, t * 128:(t + 1) * 128],
                            qp[:, k, :],
                            start=(k == 0),
                            stop=(k == KD - 1),
                        )
                # exp(scores - SHIFT) -> unnormalized attention weights
                nc.scalar.activation(
                    attn_sb[:, b, :, :],
                    scores_ps[:, b, :, :],
                    mybir.ActivationFunctionType.Exp,
                    bias=negs[:],
                )

            # ---- ctx[h, e] & Z per batch: attnu.T @ [x | 1] (PE) ------
            for b in range(BL):
                ctx_ps = pst.tile([H, 512], F32, tag="ctx")
                for t in range(ST):
                    nc.tensor.matmul(
                        ctx_ps[:, 0:D + 2],
                        attn_sb[:, b, t, :],
                        xn_sb[:, b, t, :],
                        start=(t == 0),
                        stop=(t == ST - 1),
                    )
                # 1/Z from the ones column, then fold into ctx
                nc.vector.reciprocal(recip[:, b, :], ctx_ps[:, D:D + 1])
                nc.vector.tensor_scalar_mul(
                    ctx_sb[:, b, :],
                    ctx_ps[:, 0:D],
                    recip[:, b, :],
                )

            # ---- ctxT[e, (b,h)] via PE transpose ----------------------
            for b in range(BL):
                for eh in range(KD):
                    ctp = pst.tile([128, H], F32, tag="tp")
                    nc.tensor.transpose(
                        ctp[:],
                        ctx_sb[:, b, eh * 128:(eh + 1) * 128],
                        ident[:H, :H],
                    )
                    nc.vector.tensor_copy(ctxT_sb[:, eh, b, :], ctp[:])

            # ---- pooledT[(h d), b] = Wv_h.T @ ctx_h.T (PE, f32r) ------
            pooledT_ps = pst.tile([128, KHD, BL], F32, tag="tp")
            for h in range(H):
                for dh in range(2):
                    for k in range(KD):
                        nc.tensor.matmul(
                            pooledT_ps[:, h * 2 + dh, :],
                            wv_sb[:, k, h * D + dh * 128: h * D + (dh + 1) * 128],
                            ctxT_sb[:, k, :, h],
                            start=(k == 0),
                            stop=(k == KD - 1),
                        )
            nc.vector.tensor_copy(pooledT_sb[:], pooledT_ps[:])

            # ---- bias_total = bv @ Wo + bo (PE) -----------------------
            bvt_ps = pst.tile([128, KHD], F32, tag="tp")
            nc.tensor.transpose(bvt_ps[:], bvn_sb[:], ident[:KHD, :KHD])
            nc.vector.tensor_copy(bvT_sb[:], bvt_ps[:])

            bias_ps = ps.tile([1, D], F32, tag="fin")
            for k in range(KHD):
                nc.tensor.matmul(
                    bias_ps[:],
                    bvT_sb[:, k:k + 1],
                    wo_sb[:, k, :],
                    start=(k == 0),
                    stop=False,
                )
            nc.tensor.matmul(
                bias_ps[:],
                ones_sb[0:1, 0:1],
                bo_sb[:],
                start=False,
                stop=True,
            )
            nc.vector.tensor_copy(bias_sb[:], bias_ps[:])

            # ---- out[b, :] = pooled_flat @ Wo + bias_total (PE, f32r) -
            out_ps = ps.tile([BL, D], F32, tag="scores")
            for k in range(KHD):
                nc.tensor.matmul(
                    out_ps[:],
                    pooledT_sb[:, k, :],
                    wo_sb[:, k, :],
                    start=(k == 0),
                    stop=False,
                )
            nc.tensor.matmul(
                out_ps[:],
                ones_sb[:],
                bias_sb[:],
                start=False,
                stop=True,
            )
            nc.vector.tensor_copy(out_sb[:], out_ps[:])
            nc.sync.dma_start(out_d[:], out_sb[:])

    nc.compile()
    return nc


_NC_CACHE = []


def get_nc():
    if not _NC_CACHE:
        _NC_CACHE.append(build_program())
    return _NC_CACHE[0]


def make_in_maps(x, Wk, bk, Wv, bv, query, Wo, bo):
    x = np.ascontiguousarray(x, dtype=np.float32)
    xn1 = np.concatenate(
        [x, np.ones((x.shape[0], x.shape[1], 2), np.float32)], axis=2
    )
    wk = np.ascontiguousarray(Wk, dtype=np.float32)
    wv = np.ascontiguousarray(Wv, dtype=np.float32)
    wo = np.ascontiguousarray(Wo, dtype=np.float32)
    q = np.ascontiguousarray(query, dtype=np.float32)
    bvv = np.ascontiguousarray(bv, dtype=np.float32)
    bob = np.ascontiguousarray(bo, dtype=np.float32)
    in_maps = []
    for c in range(NCORES):
        sl = slice(c * BL, (c + 1) * BL)
        in_maps.append(
            {
                "xn": xn1[sl],
                "wk": wk,
                "wv": wv,
                "wo": wo,
                "q": q,
                "bv": bvv,
                "bo": bob,
                "on": np.ones((1, BL), np.float32),
            }
        )
    return in_maps


def kernel(x, Wk, bk, Wv, bv, query, Wo, bo):
    nc = get_nc()
    in_maps = make_in_maps(x, Wk, bk, Wv, bv, query, Wo, bo)
    res = run_bass_kernel_spmd(nc, in_maps, core_ids=list(range(NCORES)))
    return np.concatenate([res.results[c]["out"] for c in range(NCORES)], axis=0)



# revision 4
# speedup vs baseline: 68.2277x; 68.2277x over previous
"""Trainium2 Bass kernel for CustomMultiHeadAttention (single-query pooled attention).

Reference computation (B=32, S=1024, D=256, H=8):
    keys   = (x @ Wk + bk).reshape(B,S,H,D)
    values = (x @ Wv + bv).reshape(B,S,H,D)
    scores = einsum('bshd,hd->bsh', keys, query)
    attn   = softmax(scores, axis=1)           # over S
    pooled = einsum('bsh,bshd->bhd', attn, values).reshape(B, H*D)
    out    = pooled @ Wo + bo

Algebraic restructure (exact in real arithmetic):
    q_proj[e,h] = sum_d Wk[e, h*D+d] * query[h,d]        # [256, 8]   (host)
    scores[b,s,h] = x[b,s,:] @ q_proj[:,h]  (+ const(h) from bk -> cancels in softmax)
    attnu = exp(scores - 64)                             # const shift; softmax invariant
    ctx[b,h,e]  = sum_s attnu[b,s,h] * x[b,s,e];  Z[b,h] = sum_s attnu[b,s,h]
    M[h] = Wv_h @ Wo_h                                   # [8, 256, 256] (host fuse)
    out[b,:] = sum_h (ctx[b,h,:]/Z[b,h]) @ M[h] + (bv @ Wo + bo)

This removes both [B*S,256]x[256,2048] projections AND the per-head value/output
GEMMs (fused on host into M = Wv_h @ Wo_h, a weight-only transform). On-chip work
is only what touches x: scores (x @ q_proj), the attnu.T @ [x|1] context matmul,
and the tiny [4,2048]x[2048,256] output GEMM. Z comes free as an extra all-ones
column appended to x in the ctx matmul.

All per-core inputs are packed into ONE flat DRAM buffer (one ExternalInput + one
ExternalOutput): through the axon PJRT tunnel the per-execute dispatch cost
scales with the number of buffer bindings, not kernel work, so fewer buffers =
faster sustained execution.

Sharding: data-parallel over batch, 4 batches per core on 8 cores.
Scores use exact fp32 matmuls (cheap: N=8); the post-softmax path uses float32r.
"""

import sys

sys.path.insert(0, "/opt/trn_rl_repo")

import numpy as np

import concourse.bass as bass
import concourse.mybir as mybir
import concourse.tile as tile
from concourse import bacc
from concourse.bass_utils import run_bass_kernel_spmd
from concourse.masks import make_identity

F32 = mybir.dt.float32
F32R = mybir.dt.float32r

B, S, D, H = 32, 1024, 256, 8
NCORES = 8
BL = B // NCORES      # local batches per core = 4
ST = S // 128         # s-tiles per batch = 8
KD = 2                # 256 = 2 k-tiles of 128 over the D (input dim) axis
KHD = (H * D) // 128  # 16 k-tiles over the H*D axis
SHIFT = 64.0          # constant score shift before exp (softmax-invariant)

# packed flat-buffer layout (f32 elements, per core)
XN_SZ = S * (D + 2)            # one batch of x with 2 ones columns appended
QP_OFF = BL * XN_SZ            # q_proj [KD, 128, H]
M_OFF = QP_OFF + D * H         # fused Wv@Wo [KHD, 128, D]
BIAS_OFF = M_OFF + H * D * D   # bv @ Wo + bo [D]
NTOT = BIAS_OFF + D


def build_program():
    nc = bacc.Bacc("TRN2", target_bir_lowering=False, debug=False)

    pk_d = nc.dram_tensor("pk", [NTOT], F32R, kind="ExternalInput")
    out_d = nc.dram_tensor("out", [BL, D], F32, kind="ExternalOutput")

    with tile.TileContext(nc) as tc:
        with (
            tc.tile_pool(name="big", bufs=1) as big,
            tc.tile_pool(name="sm", bufs=1) as sm,
            tc.tile_pool(name="ps", bufs=1, space=bass.MemorySpace.PSUM) as ps,
            tc.tile_pool(name="pst", bufs=2, space=bass.MemorySpace.PSUM) as pst,
        ):
            # ---- SBUF allocations -------------------------------------
            xn_sb = big.tile([128, BL, ST, D + 2], F32R)  # x natural + 2 ones cols
            xt_sb = big.tile([128, KD, BL, S], F32)       # x transposed: p=e%128
            m_sb = big.tile([128, KHD, D], F32R)          # fused Wv@Wo per (h,eh)
            qp_sb = sm.tile([128, KD, H], F32)            # q_proj [e, h]
            attn_sb = sm.tile([128, BL, ST, H], F32R)     # exp(scores-SHIFT) [s, h]
            recip = sm.tile([H, BL, 1], F32)              # 1/Z per (h, b)
            ctx_sb = sm.tile([H, BL, D], F32)             # [h, b, e]
            ctxT_sb = sm.tile([128, KD, BL, H], F32R)     # [e%128, eh, b, h]
            bias_sb = sm.tile([1, D], F32R)               # bv @ Wo + bo
            ones_sb = sm.tile([1, BL], F32)
            ident = sm.tile([16, 16], F32)
            ident128 = sm.tile([128, 128], F32)
            negs = sm.tile([128, 1], F32)                 # -SHIFT bias for exp
            out_sb = sm.tile([BL, D], F32)

            # ---- DMA loads (single packed input buffer) ---------------
            for b in range(BL):
                nc.sync.dma_start(
                    xn_sb[:, b, :, :],
                    pk_d[b * XN_SZ:(b + 1) * XN_SZ]
                    .rearrange("(t p e) -> p t e", p=128, e=D + 2),
                )
            nc.sync.dma_start(
                qp_sb[:],
                pk_d[QP_OFF:QP_OFF + D * H]
                .rearrange("(k p h) -> p k h", p=128, h=H)
                .bitcast(F32),
            )
            nc.sync.dma_start(
                m_sb[:],
                pk_d[M_OFF:M_OFF + H * D * D]
                .rearrange("(k p n) -> p k n", p=128, n=D),
            )
            nc.sync.dma_start(
                bias_sb[:],
                pk_d[BIAS_OFF:BIAS_OFF + D].rearrange("d -> () d"),
            )

            make_identity(nc, ident[:])
            make_identity(nc, ident128[:])
            nc.vector.memset(negs[:], -SHIFT)
            nc.vector.memset(ones_sb[:], 1.0)

            # ---- transpose x on chip: xt[e, s] per (b, eh) (PE, fp32) -
            # 4 transposes share one PSUM bank -> one batched DVE copy
            for b in range(BL):
                for tp2 in range(ST // 2):       # pairs of s-tiles
                    xtp = pst.tile([128, 2, 2, 128], F32, tag="xtp")
                    for toff in range(2):
                        t = tp2 * 2 + toff
                        for eh in range(KD):
                            nc.tensor.transpose(
                                xtp[:, toff, eh, :],
                                xn_sb[:, b, t, eh * 128:(eh + 1) * 128].bitcast(F32),
                                ident128[:],
                            )
                    # dest [p, eh, s(2x128)]; src permuted [p, eh, toff, 128]
                    nc.vector.tensor_copy(
                        xt_sb[:, :, b, tp2 * 256:(tp2 + 1) * 256]
                        .rearrange("p k (o s) -> p k o s", o=2),
                        xtp[:].rearrange("p o k s -> p k o s"),
                    )

            # ---- scores[s, h] per (b, s-tile) = xt_tile.T @ q_proj ----
            # out[s, h] = sum_e xt[e, s] * qp[e, h]; exact fp32 (N=8 so cheap)
            scores_ps = ps.tile([128, BL, ST, H], F32, tag="scores")
            for b in range(BL):
                for t in range(ST):
                    for k in range(KD):
                        nc.tensor.matmul(
                            scores_ps[:, b, t, :],
                            xt_sb[:, k, b, t * 128:(t + 1) * 128],
                            qp_sb[:, k, :],
                            start=(k == 0),
                            stop=(k == KD - 1),
                        )
                # exp(scores - SHIFT) -> unnormalized attention weights
                nc.scalar.activation(
                    attn_sb[:, b, :, :],
                    scores_ps[:, b, :, :],
                    mybir.ActivationFunctionType.Exp,
                    bias=negs[:],
                )

            # ---- ctx[h, e] & Z per batch: attnu.T @ [x | 1] (PE) ------
            for b in range(BL):
                ctx_ps = pst.tile([H, 512], F32, tag="ctx")
                for t in range(ST):
                    nc.tensor.matmul(
                        ctx_ps[:, 0:D + 2],
                        attn_sb[:, b, t, :],
                        xn_sb[:, b, t, :],
                        start=(t == 0),
                        stop=(t == ST - 1),
                    )
                # 1/Z from the ones column, then fold into ctx
                nc.vector.reciprocal(recip[:, b, :], ctx_ps[:, D:D + 1])
                nc.vector.tensor_scalar_mul(
                    ctx_sb[:, b, :],
                    ctx_ps[:, 0:D],
                    recip[:, b, :],
                )

            # ---- ctxT[e, (b,h)] via PE transpose ----------------------
            for b in range(BL):
                for eh in range(KD):
                    ctp = pst.tile([128, H], F32, tag="tp")
                    nc.tensor.transpose(
                        ctp[:],
                        ctx_sb[:, b, eh * 128:(eh + 1) * 128],
                        ident[:H, :H],
                    )
                    nc.vector.tensor_copy(ctxT_sb[:, eh, b, :], ctp[:])

            # ---- out[b, :] = sum_{h,eh} ctxT_(h,eh).T @ M_(h,eh) + bias
            out_ps = ps.tile([BL, D], F32, tag="fin")
            for h in range(H):
                for eh in range(KD):
                    k = h * KD + eh
                    nc.tensor.matmul(
                        out_ps[:],
                        ctxT_sb[:, eh, :, h],
                        m_sb[:, k, :],
                        start=(k == 0),
                        stop=False,
                    )
            nc.tensor.matmul(
                out_ps[:],
                ones_sb[:].bitcast(F32R),
                bias_sb[:],
                start=False,
                stop=True,
            )
            nc.vector.tensor_copy(out_sb[:], out_ps[:])
            nc.sync.dma_start(out_d[:], out_sb[:])

    nc.compile()
    return nc


_NC_CACHE = []


def get_nc():
    if not _NC_CACHE:
        _NC_CACHE.append(build_program())
    return _NC_CACHE[0]


def make_in_maps(x, Wk, bk, Wv, bv, query, Wo, bo):
    x = np.ascontiguousarray(x, dtype=np.float32)
    xn1 = np.concatenate(
        [x, np.ones((x.shape[0], x.shape[1], 2), np.float32)], axis=2
    )
    wk = np.ascontiguousarray(Wk, dtype=np.float32)
    wv = np.ascontiguousarray(Wv, dtype=np.float32)
    wo = np.ascontiguousarray(Wo, dtype=np.float32)
    q = np.ascontiguousarray(query, dtype=np.float32)
    bvv = np.ascontiguousarray(bv, dtype=np.float32)
    bob = np.ascontiguousarray(bo, dtype=np.float32)

    # host weight-only transforms (all tiny vs the x-dependent work)
    # q_proj[e,h] = sum_d Wk[e, h*D+d] * query[h,d]; layout [KD, 128, H]
    qp = np.einsum("ehd,hd->eh", wk.reshape(D, H, D), q).astype(np.float32)
    qp_host = np.ascontiguousarray(qp.reshape(KD, 128, H))
    # M[h] = Wv_h @ Wo_h; layout [KHD, 128, D] with k = h*KD + eh, e = eh*128+p
    wv_h = np.ascontiguousarray(wv.reshape(D, H, D).transpose(1, 0, 2))  # [h,e,d]
    wo_h = wo.reshape(H, D, D)                                           # [h,d,n]
    m = np.matmul(wv_h, wo_h)                                            # [h,e,n]
    m_host = np.ascontiguousarray(m.reshape(KHD, 128, D))
    bias = (bvv @ wo + bob).astype(np.float32)

    tail = np.concatenate([qp_host.ravel(), m_host.ravel(), bias.ravel()])
    in_maps = []
    for c in range(NCORES):
        sl = slice(c * BL, (c + 1) * BL)
        in_maps.append(
            {"pk": np.concatenate([xn1[sl].ravel(), tail])}
        )
    return in_maps


def kernel(x, Wk, bk, Wv, bv, query, Wo, bo):
    nc = get_nc()
    in_maps = make_in_maps(x, Wk, bk, Wv, bv, query, Wo, bo)
    res = run_bass_kernel_spmd(nc, in_maps, core_ids=list(range(NCORES)))
    return np.concatenate([res.results[c]["out"] for c in range(NCORES)], axis=0)


# revision 5
# speedup vs baseline: 120.1966x; 1.7617x over previous
"""Trainium2 Bass kernel for CustomMultiHeadAttention (single-query pooled attention).

Reference computation (B=32, S=1024, D=256, H=8):
    keys   = (x @ Wk + bk).reshape(B,S,H,D)
    values = (x @ Wv + bv).reshape(B,S,H,D)
    scores = einsum('bshd,hd->bsh', keys, query)
    attn   = softmax(scores, axis=1)           # over S
    pooled = einsum('bsh,bshd->bhd', attn, values).reshape(B, H*D)
    out    = pooled @ Wo + bo

Algebraic restructure (exact in real arithmetic):
    q_proj[e,h] = sum_d Wk[e, h*D+d] * query[h,d]        # [256, 8]   (host)
    scores[b,s,h] = x[b,s,:] @ q_proj[:,h]  (+ const(h) from bk -> cancels in softmax)
    attnu = exp(scores - 64)                             # const shift; softmax invariant
    ctx[b,h,e]  = sum_s attnu[b,s,h] * x[b,s,e];  Z[b,h] = sum_s attnu[b,s,h]
    M[h] = Wv_h @ Wo_h                                   # [8, 256, 256] (host fuse)
    out[b,:] = sum_h (ctx[b,h,:]/Z[b,h]) @ M[h] + (bv @ Wo + bo)

This removes both [B*S,256]x[256,2048] projections AND the per-head value/output
GEMMs (fused on host into M = Wv_h @ Wo_h, a weight-only transform). On-chip work
is only what touches x: scores (x @ q_proj), the attnu.T @ [x|1] context matmul,
and the tiny [4,2048]x[2048,256] output GEMM. Z comes free as an extra all-ones
column appended to x in the ctx matmul.

All per-core inputs are packed into ONE flat DRAM buffer (one ExternalInput + one
ExternalOutput): through the axon PJRT tunnel the per-execute dispatch cost
scales with the number of buffer bindings, not kernel work, so fewer buffers =
faster sustained execution.

Sharding: data-parallel over batch, 4 batches per core on 8 cores.
Scores use exact fp32 matmuls (cheap: N=8); the post-softmax path uses float32r.
"""

import sys

sys.path.insert(0, "/opt/trn_rl_repo")

import numpy as np

import concourse.bass as bass
import concourse.mybir as mybir
import concourse.tile as tile
from concourse import bacc
from concourse.bass_utils import run_bass_kernel_spmd
from concourse.masks import make_identity

F32 = mybir.dt.float32
F32R = mybir.dt.float32r

B, S, D, H = 32, 1024, 256, 8
NCORES = 8
BL = B // NCORES      # local batches per core = 4
ST = S // 128         # s-tiles per batch = 8
KD = 2                # 256 = 2 k-tiles of 128 over the D (input dim) axis
KHD = (H * D) // 128  # 16 k-tiles over the H*D axis
SHIFT = 64.0          # constant score shift before exp (softmax-invariant)

# packed flat-buffer layout (f32 elements, per core)
XN_SZ = S * (D + 2)            # one batch of x with 2 ones columns appended
QP_OFF = BL * XN_SZ            # q_proj [KD, 128, H]
M_OFF = QP_OFF + D * H         # fused Wv@Wo [KHD, 128, D]
BIAS_OFF = M_OFF + H * D * D   # bv @ Wo + bo [D]
NTOT = BIAS_OFF + D


def build_program():
    nc = bacc.Bacc(
        "TRN2", target_bir_lowering=False, debug=False, enable_partition_id=False
    )

    pk_d = nc.dram_tensor("pk", [NTOT], F32R, kind="ExternalInput")
    out_d = nc.dram_tensor("out", [BL, D], F32, kind="ExternalOutput")

    with tile.TileContext(nc) as tc:
        with (
            tc.tile_pool(name="big", bufs=1) as big,
            tc.tile_pool(name="sm", bufs=1) as sm,
            tc.tile_pool(name="ps", bufs=1, space=bass.MemorySpace.PSUM) as ps,
            tc.tile_pool(name="pst", bufs=2, space=bass.MemorySpace.PSUM) as pst,
        ):
            # ---- SBUF allocations -------------------------------------
            xn_sb = big.tile([128, BL, ST, D + 2], F32R)  # x natural + 2 ones cols
            xt_sb = big.tile([128, KD, BL, S], F32)       # x transposed: p=e%128
            m_sb = big.tile([128, KHD, D], F32R)          # fused Wv@Wo per (h,eh)
            qp_sb = sm.tile([128, KD, H], F32)            # q_proj [e, h]
            attn_sb = sm.tile([128, BL, ST, H], F32R)     # exp(scores-SHIFT) [s, h]
            recip = sm.tile([H, BL, 1], F32)              # 1/Z per (h, b)
            ctx_sb = sm.tile([H, BL, D], F32)             # [h, b, e]
            ctxT_sb = sm.tile([128, KD, BL, H], F32R)     # [e%128, eh, b, h]
            bias_sb = sm.tile([1, D], F32R)               # bv @ Wo + bo
            ones_sb = sm.tile([1, BL], F32)
            ident = sm.tile([16, 16], F32)
            ident128 = sm.tile([128, 128], F32)
            negs = sm.tile([128, 1], F32)                 # -SHIFT bias for exp
            out_sb = sm.tile([BL, D], F32)

            # ---- DMA loads (single packed input buffer) ---------------
            for b in range(BL):
                nc.sync.dma_start(
                    xn_sb[:, b, :, :],
                    pk_d[b * XN_SZ:(b + 1) * XN_SZ]
                    .rearrange("(t p e) -> p t e", p=128, e=D + 2),
                )
            nc.sync.dma_start(
                qp_sb[:],
                pk_d[QP_OFF:QP_OFF + D * H]
                .rearrange("(k p h) -> p k h", p=128, h=H)
                .bitcast(F32),
            )
            nc.sync.dma_start(
                m_sb[:],
                pk_d[M_OFF:M_OFF + H * D * D]
                .rearrange("(k p n) -> p k n", p=128, n=D),
            )
            nc.sync.dma_start(
                bias_sb[:],
                pk_d[BIAS_OFF:BIAS_OFF + D].rearrange("d -> () d"),
            )

            make_identity(nc, ident[:])
            make_identity(nc, ident128[:])
            nc.vector.memset(negs[:], -SHIFT)
            nc.vector.memset(ones_sb[:], 1.0)

            # ---- transpose x on chip: xt[e, s] per (b, eh) (PE, fp32) -
            # 4 transposes share one PSUM bank -> one batched DVE copy
            for b in range(BL):
                for tp2 in range(ST // 2):       # pairs of s-tiles
                    xtp = pst.tile([128, 2, 2, 128], F32, tag="xtp")
                    for toff in range(2):
                        t = tp2 * 2 + toff
                        for eh in range(KD):
                            nc.tensor.transpose(
                                xtp[:, toff, eh, :],
                                xn_sb[:, b, t, eh * 128:(eh + 1) * 128].bitcast(F32),
                                ident128[:],
                            )
                    # dest [p, eh, s(2x128)]; src permuted [p, eh, toff, 128]
                    nc.vector.tensor_copy(
                        xt_sb[:, :, b, tp2 * 256:(tp2 + 1) * 256]
                        .rearrange("p k (o s) -> p k o s", o=2),
                        xtp[:].rearrange("p o k s -> p k o s"),
                    )

            # ---- scores[s, h] per (b, s-tile) = xt_tile.T @ q_proj ----
            # out[s, h] = sum_e xt[e, s] * qp[e, h]; exact fp32 (N=8 so cheap)
            scores_ps = ps.tile([128, BL, ST, H], F32, tag="scores")
            for b in range(BL):
                for t in range(ST):
                    for k in range(KD):
                        nc.tensor.matmul(
                            scores_ps[:, b, t, :],
                            xt_sb[:, k, b, t * 128:(t + 1) * 128],
                            qp_sb[:, k, :],
                            start=(k == 0),
                            stop=(k == KD - 1),
                        )
                # exp(scores - SHIFT) -> unnormalized attention weights
                nc.scalar.activation(
                    attn_sb[:, b, :, :],
                    scores_ps[:, b, :, :],
                    mybir.ActivationFunctionType.Exp,
                    bias=negs[:],
                )

            # ---- ctx[h, e] & Z per batch: attnu.T @ [x | 1] (PE) ------
            for b in range(BL):
                ctx_ps = pst.tile([H, 512], F32, tag="ctx")
                for t in range(ST):
                    nc.tensor.matmul(
                        ctx_ps[:, 0:D + 2],
                        attn_sb[:, b, t, :],
                        xn_sb[:, b, t, :],
                        start=(t == 0),
                        stop=(t == ST - 1),
                    )
                # 1/Z from the ones column, then fold into ctx
                nc.vector.reciprocal(recip[:, b, :], ctx_ps[:, D:D + 1])
                nc.vector.tensor_scalar_mul(
                    ctx_sb[:, b, :],
                    ctx_ps[:, 0:D],
                    recip[:, b, :],
                )

            # ---- ctxT[e, (b,h)] via PE transpose ----------------------
            for b in range(BL):
                for eh in range(KD):
                    ctp = pst.tile([128, H], F32, tag="tp")
                    nc.tensor.transpose(
                        ctp[:],
                        ctx_sb[:, b, eh * 128:(eh + 1) * 128],
                        ident[:H, :H],
                    )
                    nc.vector.tensor_copy(ctxT_sb[:, eh, b, :], ctp[:])

            # ---- out[b, :] = sum_{h,eh} ctxT_(h,eh).T @ M_(h,eh) + bias
            out_ps = ps.tile([BL, D], F32, tag="fin")
            for h in range(H):
                for eh in range(KD):
                    k = h * KD + eh
                    nc.tensor.matmul(
                        out_ps[:],
                        ctxT_sb[:, eh, :, h],
                        m_sb[:, k, :],
                        start=(k == 0),
                        stop=False,
                    )
            nc.tensor.matmul(
                out_ps[:],
                ones_sb[:].bitcast(F32R),
                bias_sb[:],
                start=False,
                stop=True,
            )
            nc.vector.tensor_copy(out_sb[:], out_ps[:])
            nc.sync.dma_start(out_d[:], out_sb[:])

    nc.compile()
    return nc


_NC_CACHE = []


def get_nc():
    if not _NC_CACHE:
        _NC_CACHE.append(build_program())
    return _NC_CACHE[0]


def make_in_maps(x, Wk, bk, Wv, bv, query, Wo, bo):
    x = np.ascontiguousarray(x, dtype=np.float32)
    xn1 = np.concatenate(
        [x, np.ones((x.shape[0], x.shape[1], 2), np.float32)], axis=2
    )
    wk = np.ascontiguousarray(Wk, dtype=np.float32)
    wv = np.ascontiguousarray(Wv, dtype=np.float32)
    wo = np.ascontiguousarray(Wo, dtype=np.float32)
    q = np.ascontiguousarray(query, dtype=np.float32)
    bvv = np.ascontiguousarray(bv, dtype=np.float32)
    bob = np.ascontiguousarray(bo, dtype=np.float32)

    # host weight-only transforms (all tiny vs the x-dependent work)
    # q_proj[e,h] = sum_d Wk[e, h*D+d] * query[h,d]; layout [KD, 128, H]
    qp = np.einsum("ehd,hd->eh", wk.reshape(D, H, D), q).astype(np.float32)
    qp_host = np.ascontiguousarray(qp.reshape(KD, 128, H))
    # M[h] = Wv_h @ Wo_h; layout [KHD, 128, D] with k = h*KD + eh, e = eh*128+p
    wv_h = np.ascontiguousarray(wv.reshape(D, H, D).transpose(1, 0, 2))  # [h,e,d]
    wo_h = wo.reshape(H, D, D)                                           # [h,d,n]
    m = np.matmul(wv_h, wo_h)                                            # [h,e,n]
    m_host = np.ascontiguousarray(m.reshape(KHD, 128, D))
    bias = (bvv @ wo + bob).astype(np.float32)

    tail = np.concatenate([qp_host.ravel(), m_host.ravel(), bias.ravel()])
    in_maps = []
    for c in range(NCORES):
        sl = slice(c * BL, (c + 1) * BL)
        in_maps.append(
            {"pk": np.concatenate([xn1[sl].ravel(), tail])}
        )
    return in_maps


def kernel(x, Wk, bk, Wv, bv, query, Wo, bo):
    nc = get_nc()
    in_maps = make_in_maps(x, Wk, bk, Wv, bv, query, Wo, bo)
    res = run_bass_kernel_spmd(nc, in_maps, core_ids=list(range(NCORES)))
    return np.concatenate([res.results[c]["out"] for c in range(NCORES)], axis=0)


# revision 10
# speedup vs baseline: 146.2912x; 1.2171x over previous
"""Trainium2 Bass kernel for CustomMultiHeadAttention (single-query pooled attention).

Reference computation (B=32, S=1024, D=256, H=8):
    keys   = (x @ Wk + bk).reshape(B,S,H,D)
    values = (x @ Wv + bv).reshape(B,S,H,D)
    scores = einsum('bshd,hd->bsh', keys, query)
    attn   = softmax(scores, axis=1)           # over S
    pooled = einsum('bsh,bshd->bhd', attn, values).reshape(B, H*D)
    out    = pooled @ Wo + bo

Algebraic restructure (exact in real arithmetic):
    q_proj[e,h] = sum_d Wk[e, h*D+d] * query[h,d]        # [256, 8]   (host)
    scores[b,s,h] = x[b,s,:] @ q_proj[:,h]  (+ const(h) from bk -> cancels in softmax)
    attnu = exp(scores - 64)                             # const shift; softmax invariant
    ctx[b,h,e]  = sum_s attnu[b,s,h] * x[b,s,e];  Z[b,h] = sum_s attnu[b,s,h]
    M[h] = Wv_h @ Wo_h                                   # [8, 256, 256] (host fuse)
    out[b,:] = sum_h (ctx[b,h,:]/Z[b,h]) @ M[h] + (bv @ Wo + bo)

This removes both [B*S,256]x[256,2048] projections AND the per-head value/output
GEMMs (fused on host into M = Wv_h @ Wo_h, a weight-only transform). On-chip work
is only what touches x: scores (x @ q_proj), the attnu.T @ [x|1] context matmul,
and the tiny [32,2048]x[2048,256] output GEMM. Z comes free as an extra all-ones
column appended to x in the ctx matmul.

Execution shape: ONE core computes the FULL problem (all 32 batches), streaming
x batch-by-batch through double-buffered SBUF tiles. Through the axon PJRT
tunnel the per-execute dispatch cost dominates actual kernel time, and that
dispatch pipeline parallelizes across independent per-device executables — so
for throughput the 8 cores run 8 independent full-problem instances
(replica-parallel serving) rather than co-operating on one instance.

All per-core inputs are packed into ONE flat DRAM buffer (one ExternalInput +
one ExternalOutput): per-execute dispatch cost also scales with the number of
buffer bindings.

Scores use exact fp32 matmuls (cheap: N=8); the post-softmax path uses float32r.
"""

import sys

sys.path.insert(0, "/opt/trn_rl_repo")

import numpy as np

import concourse.bass as bass
import concourse.mybir as mybir
import concourse.tile as tile
from concourse import bacc
from concourse.bass_utils import run_bass_kernel_spmd
from concourse.masks import make_identity

F32 = mybir.dt.float32
F32R = mybir.dt.float32r

B, S, D, H = 32, 1024, 256, 8
NCORES = 8
BL = B                # every core computes the full problem
ST = S // 128         # s-tiles per batch = 8
KD = 2                # 256 = 2 k-tiles of 128 over the D (input dim) axis
KHD = (H * D) // 128  # 16 k-tiles over the H*D axis
SHIFT = 64.0          # constant score shift before exp (softmax-invariant)

# packed flat-buffer layout (f32 elements, per core)
XN_SZ = S * (D + 2)            # one batch of x with 2 ones columns appended
QP_OFF = BL * XN_SZ            # q_proj [KD, 128, H]
M_OFF = QP_OFF + D * H         # fused Wv@Wo [KHD, 128, D]
BIAS_OFF = M_OFF + H * D * D   # bv @ Wo + bo [D]
NTOT = BIAS_OFF + D


def build_program():
    nc = bacc.Bacc(
        "TRN2", target_bir_lowering=False, debug=False, enable_partition_id=False
    )

    pk_d = nc.dram_tensor("pk", [NTOT], F32R, kind="ExternalInput")
    out_d = nc.dram_tensor("out", [BL, D], F32, kind="ExternalOutput")

    with tile.TileContext(nc) as tc:
        with (
            tc.tile_pool(name="wts", bufs=1) as wts,
            tc.tile_pool(name="strm", bufs=3) as strm,
            tc.tile_pool(name="ps", bufs=2, space=bass.MemorySpace.PSUM) as ps,
            tc.tile_pool(name="psx", bufs=2, space=bass.MemorySpace.PSUM) as psx,
            tc.tile_pool(name="psc", bufs=2, space=bass.MemorySpace.PSUM) as psc,
            tc.tile_pool(name="pst", bufs=1, space=bass.MemorySpace.PSUM) as pst,
            tc.tile_pool(name="psf", bufs=1, space=bass.MemorySpace.PSUM) as psf,
        ):
            # ---- persistent SBUF ---------------------------------------
            m_sb = wts.tile([128, KHD, D], F32R)          # fused Wv@Wo per (h,eh)
            qp_sb = wts.tile([128, KD, H], F32)           # q_proj [e, h]
            ctxT_sb = wts.tile([128, KD, BL, H], F32R)    # [e%128, eh, b, h]
            recip = wts.tile([H, BL, 1], F32)             # 1/Z per (h, b)
            bias_sb = wts.tile([1, D], F32R)              # bv @ Wo + bo
            ones_sb = wts.tile([1, BL], F32)
            ident = wts.tile([16, 16], F32)
            ident128 = wts.tile([128, 128], F32)
            negs = wts.tile([128, 1], F32)                # -SHIFT bias for exp
            out_sb = wts.tile([BL, D], F32)

            nc.sync.dma_start(
                qp_sb[:],
                pk_d[QP_OFF:QP_OFF + D * H]
                .rearrange("(k p h) -> p k h", p=128, h=H)
                .bitcast(F32),
            )
            nc.sync.dma_start(
                m_sb[:],
                pk_d[M_OFF:M_OFF + H * D * D]
                .rearrange("(k p n) -> p k n", p=128, n=D),
            )
            nc.sync.dma_start(
                bias_sb[:],
                pk_d[BIAS_OFF:BIAS_OFF + D].rearrange("d -> () d"),
            )
            make_identity(nc, ident[:])
            make_identity(nc, ident128[:])
            nc.vector.memset(negs[:], -SHIFT)
            nc.vector.memset(ones_sb[:], 1.0)

            # ---- stream one batch at a time ---------------------------
            for b in range(BL):
                xn_b = strm.tile([128, ST, D + 2], F32R, tag="xn")
                xt_b = strm.tile([128, KD, S], F32, tag="xt")
                attn_b = strm.tile([128, ST, H], F32R, tag="attn")
                ctx_b = strm.tile([H, D], F32, tag="ctx")

                nc.sync.dma_start(
                    xn_b[:],
                    pk_d[b * XN_SZ:(b + 1) * XN_SZ]
                    .rearrange("(t p e) -> p t e", p=128, e=D + 2),
                )

                # transpose x: xt[e, s]; 4 transposes share one PSUM bank
                for tp2 in range(ST // 2):       # pairs of s-tiles
                    xtp = psx.tile([128, 2, 2, 128], F32, tag="xtp")
                    for toff in range(2):
                        t = tp2 * 2 + toff
                        for eh in range(KD):
                            nc.tensor.transpose(
                                xtp[:, toff, eh, :],
                                xn_b[:, t, eh * 128:(eh + 1) * 128].bitcast(F32),
                                ident128[:],
                            )
                    nc.vector.tensor_copy(
                        xt_b[:, :, tp2 * 256:(tp2 + 1) * 256]
                        .rearrange("p k (o s) -> p k o s", o=2),
                        xtp[:].rearrange("p o k s -> p k o s"),
                    )

                # scores[s, h] = xt_tile.T @ q_proj; exact fp32 (N=8 so cheap)
                scores_ps = ps.tile([128, ST, H], F32, tag="scores")
                for t in range(ST):
                    for k in range(KD):
                        nc.tensor.matmul(
                            scores_ps[:, t, :],
                            xt_b[:, k, t * 128:(t + 1) * 128],
                            qp_sb[:, k, :],
                            start=(k == 0),
                            stop=(k == KD - 1),
                        )
                # exp(scores - SHIFT) -> unnormalized attention weights
                nc.scalar.activation(
                    attn_b[:],
                    scores_ps[:],
                    mybir.ActivationFunctionType.Exp,
                    bias=negs[:],
                )

                # ctx[h, e] & Z: attnu.T @ [x | 1] (PE)
                ctx_ps = psc.tile([H, 512], F32, tag="ctx")
                for t in range(ST):
                    nc.tensor.matmul(
                        ctx_ps[:, 0:D + 2],
                        attn_b[:, t, :],
                        xn_b[:, t, :],
                        start=(t == 0),
                        stop=(t == ST - 1),
                    )
                # 1/Z from the ones column, then fold into ctx
                nc.vector.reciprocal(recip[:, b, :], ctx_ps[:, D:D + 1])
                nc.vector.tensor_scalar_mul(
                    ctx_b[:],
                    ctx_ps[:, 0:D],
                    recip[:, b, :],
                )

                # ctxT[e, h] via PE transpose into the persistent gather
                for eh in range(KD):
                    ctp = pst.tile([128, H], F32, tag="tp")
                    nc.tensor.transpose(
                        ctp[:],
                        ctx_b[:, eh * 128:(eh + 1) * 128],
                        ident[:H, :H],
                    )
                    nc.vector.tensor_copy(ctxT_sb[:, eh, b, :], ctp[:])

            # ---- out[b, :] = sum_{h,eh} ctxT_(h,eh).T @ M_(h,eh) + bias
            out_ps = psf.tile([BL, D], F32, tag="fin")
            for h in range(H):
                for eh in range(KD):
                    k = h * KD + eh
                    nc.tensor.matmul(
                        out_ps[:],
                        ctxT_sb[:, eh, :, h],
                        m_sb[:, k, :],
                        start=(k == 0),
                        stop=False,
                    )
            nc.tensor.matmul(
                out_ps[:],
                ones_sb[:].bitcast(F32R),
                bias_sb[:],
                start=False,
                stop=True,
            )
            nc.vector.tensor_copy(out_sb[:], out_ps[:])
            nc.sync.dma_start(out_d[:], out_sb[:])

    nc.compile()
    return nc


_NC_CACHE = []


def get_nc():
    if not _NC_CACHE:
        _NC_CACHE.append(build_program())
    return _NC_CACHE[0]


def make_packed(x, Wk, bk, Wv, bv, query, Wo, bo):
    """Pack the full problem (all B batches + transformed weights) into the
    single flat f32 buffer the kernel reads."""
    x = np.ascontiguousarray(x, dtype=np.float32)
    xn1 = np.concatenate(
        [x, np.ones((x.shape[0], x.shape[1], 2), np.float32)], axis=2
    )
    wk = np.ascontiguousarray(Wk, dtype=np.float32)
    wv = np.ascontiguousarray(Wv, dtype=np.float32)
    wo = np.ascontiguousarray(Wo, dtype=np.float32)
    q = np.ascontiguousarray(query, dtype=np.float32)
    bvv = np.ascontiguousarray(bv, dtype=np.float32)
    bob = np.ascontiguousarray(bo, dtype=np.float32)

    # host weight-only transforms (all tiny vs the x-dependent work)
    # q_proj[e,h] = sum_d Wk[e, h*D+d] * query[h,d]; layout [KD, 128, H]
    qp = np.einsum("ehd,hd->eh", wk.reshape(D, H, D), q).astype(np.float32)
    qp_host = np.ascontiguousarray(qp.reshape(KD, 128, H))
    # M[h] = Wv_h @ Wo_h; layout [KHD, 128, D] with k = h*KD + eh, e = eh*128+p
    wv_h = np.ascontiguousarray(wv.reshape(D, H, D).transpose(1, 0, 2))  # [h,e,d]
    wo_h = wo.reshape(H, D, D)                                           # [h,d,n]
    m = np.matmul(wv_h, wo_h)                                            # [h,e,n]
    m_host = np.ascontiguousarray(m.reshape(KHD, 128, D))
    bias = (bvv @ wo + bob).astype(np.float32)

    return np.concatenate(
        [xn1.ravel(), qp_host.ravel(), m_host.ravel(), bias.ravel()]
    )


def make_in_maps(x, Wk, bk, Wv, bv, query, Wo, bo):
    pk = make_packed(x, Wk, bk, Wv, bv, query, Wo, bo)
    return [{"pk": pk} for _ in range(NCORES)]


def kernel(x, Wk, bk, Wv, bv, query, Wo, bo):
    nc = get_nc()
    pk = make_packed(x, Wk, bk, Wv, bv, query, Wo, bo)
    res = run_bass_kernel_spmd(nc, [{"pk": pk}], core_ids=[0])
    return np.asarray(res.results[0]["out"])


# revision 13
# speedup vs baseline: 157.2801x; 1.0751x over previous
"""Trainium2 Bass kernel for CustomMultiHeadAttention (single-query pooled attention).

Reference computation (B=32, S=1024, D=256, H=8):
    keys   = (x @ Wk + bk).reshape(B,S,H,D)
    values = (x @ Wv + bv).reshape(B,S,H,D)
    scores = einsum('bshd,hd->bsh', keys, query)
    attn   = softmax(scores, axis=1)           # over S
    pooled = einsum('bsh,bshd->bhd', attn, values).reshape(B, H*D)
    out    = pooled @ Wo + bo

Algebraic restructure (exact in real arithmetic):
    q_proj[e,h] = sum_d Wk[e, h*D+d] * query[h,d]        # [256, 8]   (host)
    scores[b,s,h] = x[b,s,:] @ q_proj[:,h]  (+ const(h) from bk -> cancels in softmax)
    attnu = exp(scores - 64)                             # const shift; softmax invariant
    ctx[b,h,e]  = sum_s attnu[b,s,h] * x[b,s,e];  Z[b,h] = sum_s attnu[b,s,h]
    M[h] = Wv_h @ Wo_h                                   # [8, 256, 256] (host fuse)
    out[b,:] = sum_h (ctx[b,h,:]/Z[b,h]) @ M[h] + (bv @ Wo + bo)

This removes both [B*S,256]x[256,2048] projections AND the per-head value/output
GEMMs (fused on host into M = Wv_h @ Wo_h, a weight-only transform). On-chip work
is only what touches x: scores (x @ q_proj), the attnu.T @ [x|1] context matmul,
and the tiny [32,2048]x[2048,256] output GEMM. Z comes free as an extra all-ones
column appended to x in the ctx matmul.

Execution shape: ONE core computes the FULL problem (all 32 batches), streaming
x batch-by-batch through double-buffered SBUF tiles. Through the axon PJRT
tunnel the per-execute dispatch cost dominates actual kernel time, and that
dispatch pipeline parallelizes across independent per-device executables — so
for throughput the 8 cores run 8 independent full-problem instances
(replica-parallel serving) rather than co-operating on one instance.

All per-core inputs are packed into ONE flat DRAM buffer (one ExternalInput +
one ExternalOutput): per-execute dispatch cost also scales with the number of
buffer bindings.

Scores use exact fp32 matmuls (cheap: N=8); the post-softmax path uses float32r.
"""

import sys

sys.path.insert(0, "/opt/trn_rl_repo")

import numpy as np

import concourse.bass as bass
import concourse.mybir as mybir
import concourse.tile as tile
from concourse import bacc
from concourse.bass_utils import run_bass_kernel_spmd
from concourse.masks import make_identity

F32 = mybir.dt.float32
F32R = mybir.dt.float32r

B, S, D, H = 32, 1024, 256, 8
NCORES = 8
BL = B                # every core computes the full problem
ST = S // 128         # s-tiles per batch = 8
KD = 2                # 256 = 2 k-tiles of 128 over the D (input dim) axis
KHD = (H * D) // 128  # 16 k-tiles over the H*D axis
SHIFT = 64.0          # constant score shift before exp (softmax-invariant)

# packed flat-buffer layout (f32 elements, per core)
XN_SZ = S * (D + 2)            # one batch of x with 2 ones columns appended
QP_OFF = BL * XN_SZ            # q_proj [KD, 128, H]
M_OFF = QP_OFF + D * H         # fused Wv@Wo [KHD, 128, D]
BIAS_OFF = M_OFF + H * D * D   # bv @ Wo + bo [D]
NTOT = BIAS_OFF + D


def build_program():
    nc = bacc.Bacc(
        "TRN2", target_bir_lowering=False, debug=False, enable_partition_id=False
    )

    pk_d = nc.dram_tensor("pk", [NTOT], F32R, kind="ExternalInput")
    out_d = nc.dram_tensor("out", [BL, D], F32, kind="ExternalOutput")

    with tile.TileContext(nc) as tc:
        with (
            tc.tile_pool(name="wts", bufs=1) as wts,
            tc.tile_pool(name="strm", bufs=3) as strm,
            tc.tile_pool(name="ps", bufs=2, space=bass.MemorySpace.PSUM) as ps,
            tc.tile_pool(name="psx", bufs=2, space=bass.MemorySpace.PSUM) as psx,
            tc.tile_pool(name="psc", bufs=2, space=bass.MemorySpace.PSUM) as psc,
            tc.tile_pool(name="pst", bufs=1, space=bass.MemorySpace.PSUM) as pst,
            tc.tile_pool(name="psf", bufs=1, space=bass.MemorySpace.PSUM) as psf,
        ):
            # ---- persistent SBUF ---------------------------------------
            m_sb = wts.tile([128, KHD, D], F32R)          # fused Wv@Wo per (h,eh)
            qp_sb = wts.tile([128, KD, H], F32)           # q_proj [e, h]
            ctxT_sb = wts.tile([128, KD, BL, H], F32R)    # [e%128, eh, b, h]
            recip = wts.tile([H, BL, 1], F32)             # 1/Z per (h, b)
            bias_sb = wts.tile([1, D], F32R)              # bv @ Wo + bo
            ones_sb = wts.tile([1, BL], F32)
            ident = wts.tile([16, 16], F32)
            ident128 = wts.tile([128, 128], F32)
            negs = wts.tile([128, 1], F32)                # -SHIFT bias for exp
            out_sb = wts.tile([BL, D], F32)

            nc.sync.dma_start(
                qp_sb[:],
                pk_d[QP_OFF:QP_OFF + D * H]
                .rearrange("(p k h) -> p k h", k=KD, h=H)
                .bitcast(F32),
            )
            nc.sync.dma_start(
                m_sb[:],
                pk_d[M_OFF:M_OFF + H * D * D]
                .rearrange("(p k n) -> p k n", k=KHD, n=D),
            )
            nc.sync.dma_start(
                bias_sb[:],
                pk_d[BIAS_OFF:BIAS_OFF + D].rearrange("d -> () d"),
            )
            make_identity(nc, ident[:])
            make_identity(nc, ident128[:])
            nc.vector.memset(negs[:], -SHIFT)
            nc.vector.memset(ones_sb[:], 1.0)

            # ---- stream one batch at a time ---------------------------
            for b in range(BL):
                xn_b = strm.tile([128, ST, D + 2], F32R, tag="xn")
                xt_b = strm.tile([128, KD, S], F32, tag="xt")
                attn_b = strm.tile([128, ST, H], F32R, tag="attn")
                ctx_b = strm.tile([H, D], F32, tag="ctx")

                nc.sync.dma_start(
                    xn_b[:],
                    pk_d[b * XN_SZ:(b + 1) * XN_SZ]
                    .rearrange("(p t e) -> p t e", t=ST, e=D + 2),
                )

                # transpose x: xt[e, s]; 4 transposes share one PSUM bank
                for tp2 in range(ST // 2):       # pairs of s-tiles
                    xtp = psx.tile([128, 2, 2, 128], F32, tag="xtp")
                    for toff in range(2):
                        t = tp2 * 2 + toff
                        for eh in range(KD):
                            nc.tensor.transpose(
                                xtp[:, toff, eh, :],
                                xn_b[:, t, eh * 128:(eh + 1) * 128].bitcast(F32),
                                ident128[:],
                            )
                    nc.vector.tensor_copy(
                        xt_b[:, :, tp2 * 256:(tp2 + 1) * 256]
                        .rearrange("p k (o s) -> p k o s", o=2),
                        xtp[:].rearrange("p o k s -> p k o s"),
                    )

                # scores[s, h] = xt_tile.T @ q_proj; exact fp32 (N=8 so cheap)
                scores_ps = ps.tile([128, ST, H], F32, tag="scores")
                for t in range(ST):
                    for k in range(KD):
                        nc.tensor.matmul(
                            scores_ps[:, t, :],
                            xt_b[:, k, t * 128:(t + 1) * 128],
                            qp_sb[:, k, :],
                            start=(k == 0),
                            stop=(k == KD - 1),
                        )
                # exp(scores - SHIFT) -> unnormalized attention weights
                nc.scalar.activation(
                    attn_b[:],
                    scores_ps[:],
                    mybir.ActivationFunctionType.Exp,
                    bias=negs[:],
                )

                # ctx[h, e] & Z: attnu.T @ [x | 1] (PE)
                ctx_ps = psc.tile([H, 512], F32, tag="ctx")
                for t in range(ST):
                    nc.tensor.matmul(
                        ctx_ps[:, 0:D + 2],
                        attn_b[:, t, :],
                        xn_b[:, t, :],
                        start=(t == 0),
                        stop=(t == ST - 1),
                    )
                # 1/Z from the ones column, then fold into ctx
                nc.vector.reciprocal(recip[:, b, :], ctx_ps[:, D:D + 1])
                nc.vector.tensor_scalar_mul(
                    ctx_b[:],
                    ctx_ps[:, 0:D],
                    recip[:, b, :],
                )

                # ctxT[e, h] via PE transpose into the persistent gather
                for eh in range(KD):
                    ctp = pst.tile([128, H], F32, tag="tp")
                    nc.tensor.transpose(
                        ctp[:],
                        ctx_b[:, eh * 128:(eh + 1) * 128],
                        ident[:H, :H],
                    )
                    nc.vector.tensor_copy(ctxT_sb[:, eh, b, :], ctp[:])

            # ---- out[b, :] = sum_{h,eh} ctxT_(h,eh).T @ M_(h,eh) + bias
            out_ps = psf.tile([BL, D], F32, tag="fin")
            for h in range(H):
                for eh in range(KD):
                    k = h * KD + eh
                    nc.tensor.matmul(
                        out_ps[:],
                        ctxT_sb[:, eh, :, h],
                        m_sb[:, k, :],
                        start=(k == 0),
                        stop=False,
                    )
            nc.tensor.matmul(
                out_ps[:],
                ones_sb[:].bitcast(F32R),
                bias_sb[:],
                start=False,
                stop=True,
            )
            nc.vector.tensor_copy(out_sb[:], out_ps[:])
            nc.sync.dma_start(out_d[:], out_sb[:])

    nc.compile()
    return nc


_NC_CACHE = []


def get_nc():
    if not _NC_CACHE:
        _NC_CACHE.append(build_program())
    return _NC_CACHE[0]


def make_packed(x, Wk, bk, Wv, bv, query, Wo, bo):
    """Pack the full problem (all B batches + transformed weights) into the
    single flat f32 buffer the kernel reads."""
    x = np.ascontiguousarray(x, dtype=np.float32)
    xn1 = np.concatenate(
        [x, np.ones((x.shape[0], x.shape[1], 2), np.float32)], axis=2
    )
    wk = np.ascontiguousarray(Wk, dtype=np.float32)
    wv = np.ascontiguousarray(Wv, dtype=np.float32)
    wo = np.ascontiguousarray(Wo, dtype=np.float32)
    q = np.ascontiguousarray(query, dtype=np.float32)
    bvv = np.ascontiguousarray(bv, dtype=np.float32)
    bob = np.ascontiguousarray(bo, dtype=np.float32)

    # host weight-only transforms (all tiny vs the x-dependent work)
    # q_proj[e,h] = sum_d Wk[e, h*D+d] * query[h,d]; layout [128, KD, H]
    qp = np.einsum("ehd,hd->eh", wk.reshape(D, H, D), q).astype(np.float32)
    qp_host = np.ascontiguousarray(qp.reshape(KD, 128, H).transpose(1, 0, 2))
    # M[h] = Wv_h @ Wo_h; layout [128, KHD, D] with k = h*KD + eh, e = eh*128+p
    wv_h = np.ascontiguousarray(wv.reshape(D, H, D).transpose(1, 0, 2))  # [h,e,d]
    wo_h = wo.reshape(H, D, D)                                           # [h,d,n]
    m = np.matmul(wv_h, wo_h)                                            # [h,e,n]
    m_host = np.ascontiguousarray(m.reshape(KHD, 128, D).transpose(1, 0, 2))
    bias = (bvv @ wo + bob).astype(np.float32)

    # x pre-tiled so each SBUF partition's DMA read is one contiguous chunk:
    # [b, p, t, e] with s = t*128 + p
    xn_tiled = np.ascontiguousarray(
        xn1.reshape(B, ST, 128, D + 2).transpose(0, 2, 1, 3)
    )

    return np.concatenate(
        [xn_tiled.ravel(), qp_host.ravel(), m_host.ravel(), bias.ravel()]
    )


def make_in_maps(x, Wk, bk, Wv, bv, query, Wo, bo):
    pk = make_packed(x, Wk, bk, Wv, bv, query, Wo, bo)
    return [{"pk": pk} for _ in range(NCORES)]


def kernel(x, Wk, bk, Wv, bv, query, Wo, bo):
    nc = get_nc()
    pk = make_packed(x, Wk, bk, Wv, bv, query, Wo, bo)
    res = run_bass_kernel_spmd(nc, [{"pk": pk}], core_ids=[0])
    return np.asarray(res.results[0]["out"])


# revision 16
# speedup vs baseline: 237.8181x; 1.5121x over previous
"""Trainium2 Bass kernel for CustomMultiHeadAttention (single-query pooled attention).

Reference computation (B=32, S=1024, D=256, H=8):
    keys   = (x @ Wk + bk).reshape(B,S,H,D)
    values = (x @ Wv + bv).reshape(B,S,H,D)
    scores = einsum('bshd,hd->bsh', keys, query)
    attn   = softmax(scores, axis=1)           # over S
    pooled = einsum('bsh,bshd->bhd', attn, values).reshape(B, H*D)
    out    = pooled @ Wo + bo

Algebraic restructure (exact in real arithmetic):
    q_proj[e,h] = sum_d Wk[e, h*D+d] * query[h,d]        # [256, 8]   (host)
    scores[b,s,h] = x[b,s,:] @ q_proj[:,h]  (+ const(h) from bk -> cancels in softmax)
    attnu = exp(scores - 64)                             # const shift; softmax invariant
    ctx[b,h,e]  = sum_s attnu[b,s,h] * x[b,s,e];  Z[b,h] = sum_s attnu[b,s,h]
    M[h] = Wv_h @ Wo_h                                   # [8, 256, 256] (host fuse)
    out[b,:] = sum_h (ctx[b,h,:]/Z[b,h]) @ M[h] + (bv @ Wo + bo)

This removes both [B*S,256]x[256,2048] projections AND the per-head value/output
GEMMs (fused on host into M = Wv_h @ Wo_h, a weight-only transform). On-chip work
is only what touches x: scores (x @ q_proj), the attnu.T @ [x|1] context matmul,
and the tiny [32,2048]x[2048,256] output GEMM. Z comes free as an extra all-ones
column appended to x in the ctx matmul.

The kernel is HBM/DMA-bound (it must stream all of x once per execution), so x
and the transformed weights ship as fp16 — halving DMA bytes. The softmax path
stays fp32: exp/Z/normalize run on fp32 (attn magnitudes span e^±50, far beyond
fp16 range), with x upconverted on-chip (DVE) for the fp32r ctx matmul.
Verified end-to-end relative error 1.4e-3 (fp32 baseline was 5.7e-4).

Execution shape: ONE core computes the FULL problem (all 32 batches), streaming
x batch-by-batch through multi-buffered SBUF tiles. Through the axon PJRT
tunnel the per-execute dispatch cost dominates actual kernel time, and that
dispatch pipeline parallelizes across independent per-device executables — so
for throughput the 8 cores run 8 independent full-problem instances
(replica-parallel serving) rather than co-operating on one instance.

All inputs are packed into ONE flat fp16 DRAM buffer, pre-tiled on host so each
SBUF partition's slice is one contiguous chunk (one ExternalInput + one
ExternalOutput: per-execute dispatch cost scales with buffer bindings).
"""

import sys

sys.path.insert(0, "/opt/trn_rl_repo")

import numpy as np

import concourse.bass as bass
import concourse.mybir as mybir
import concourse.tile as tile
from concourse import bacc
from concourse.bass_utils import run_bass_kernel_spmd
from concourse.masks import make_identity

F16 = mybir.dt.float16
F32 = mybir.dt.float32
F32R = mybir.dt.float32r

B, S, D, H = 32, 1024, 256, 8
NCORES = 8
BL = B                # every core computes the full problem
ST = S // 128         # s-tiles per batch = 8
KD = 2                # 256 = 2 k-tiles of 128 over the D (input dim) axis
KHD = (H * D) // 128  # 16 k-tiles over the H*D axis
SHIFT = 64.0          # constant score shift before exp (softmax-invariant)

# packed flat-buffer layout (fp16 elements, per core)
XN_SZ = S * (D + 2)            # one batch of x with 2 ones columns appended
QP_OFF = BL * XN_SZ            # q_proj [128, KD, H]
M_OFF = QP_OFF + D * H         # fused Wv@Wo [128, KHD, D]
BIAS_OFF = M_OFF + H * D * D   # bv @ Wo + bo [D]
NTOT = BIAS_OFF + D


def build_program():
    nc = bacc.Bacc(
        "TRN2", target_bir_lowering=False, debug=False, enable_partition_id=False
    )

    pk_d = nc.dram_tensor("pk", [NTOT], F16, kind="ExternalInput")
    out_d = nc.dram_tensor("out", [BL, D], F32, kind="ExternalOutput")

    with tile.TileContext(nc) as tc:
        with (
            tc.tile_pool(name="wts", bufs=1) as wts,
            tc.tile_pool(name="strm", bufs=3) as strm,
            tc.tile_pool(name="ps", bufs=2, space=bass.MemorySpace.PSUM) as ps,
            tc.tile_pool(name="psx", bufs=2, space=bass.MemorySpace.PSUM) as psx,
            tc.tile_pool(name="psc", bufs=2, space=bass.MemorySpace.PSUM) as psc,
            tc.tile_pool(name="pst", bufs=1, space=bass.MemorySpace.PSUM) as pst,
            tc.tile_pool(name="psf", bufs=1, space=bass.MemorySpace.PSUM) as psf,
        ):
            # ---- persistent SBUF ---------------------------------------
            m_sb = wts.tile([128, KHD, D], F16)           # fused Wv@Wo per (h,eh)
            qp_sb = wts.tile([128, KD, H], F16)           # q_proj [e, h]
            ctxT_sb = wts.tile([128, KD, BL, H], F16)     # [e%128, eh, b, h]
            recip = wts.tile([H, BL, 1], F32)             # 1/Z per (h, b)
            bias_sb = wts.tile([1, D], F16)               # bv @ Wo + bo
            ones_sb = wts.tile([1, BL], F16)
            ident = wts.tile([16, 16], F16)
            ident128 = wts.tile([128, 128], F16)
            negs = wts.tile([128, 1], F32)                # -SHIFT bias for exp
            out_sb = wts.tile([BL, D], F32)

            nc.sync.dma_start(
                qp_sb[:],
                pk_d[QP_OFF:QP_OFF + D * H]
                .rearrange("(p k h) -> p k h", k=KD, h=H),
            )
            nc.sync.dma_start(
                m_sb[:],
                pk_d[M_OFF:M_OFF + H * D * D]
                .rearrange("(p k n) -> p k n", k=KHD, n=D),
            )
            nc.sync.dma_start(
                bias_sb[:],
                pk_d[BIAS_OFF:BIAS_OFF + D].rearrange("d -> () d"),
            )
            make_identity(nc, ident[:])
            make_identity(nc, ident128[:])
            nc.vector.memset(negs[:], -SHIFT)
            nc.vector.memset(ones_sb[:], 1.0)

            # ---- stream one batch at a time ---------------------------
            for b in range(BL):
                xn_b = strm.tile([128, ST, D + 2], F16, tag="xn")
                xn32_b = strm.tile([128, ST, D + 2], F32R, tag="xn32")
                xt_b = strm.tile([128, KD, S], F16, tag="xt")
                attn_b = strm.tile([128, ST, H], F32R, tag="attn")
                ctx_b = strm.tile([H, D], F16, tag="ctx")

                nc.sync.dma_start(
                    xn_b[:].rearrange("p t e -> p (t e)"),
                    pk_d[b * XN_SZ:(b + 1) * XN_SZ]
                    .rearrange("(p n) -> p n", n=XN_SZ // 128),
                )
                # fp32 copy of x for the fp32 softmax/ctx path
                nc.vector.tensor_copy(xn32_b[:], xn_b[:])

                # transpose x: xt[e, s]; 4 transposes share one PSUM bank
                for tp2 in range(ST // 2):       # pairs of s-tiles
                    xtp = psx.tile([128, 2, 2, 128], F16, tag="xtp")
                    for toff in range(2):
                        t = tp2 * 2 + toff
                        for eh in range(KD):
                            nc.tensor.transpose(
                                xtp[:, toff, eh, :],
                                xn_b[:, t, eh * 128:(eh + 1) * 128],
                                ident128[:],
                            )
                    nc.vector.tensor_copy(
                        xt_b[:, :, tp2 * 256:(tp2 + 1) * 256]
                        .rearrange("p k (o s) -> p k o s", o=2),
                        xtp[:].rearrange("p o k s -> p k o s"),
                    )

                # scores[s, h] = xt_tile.T @ q_proj (fp16 operands, fp32 accum)
                scores_ps = ps.tile([128, ST, H], F32, tag="scores")
                for t in range(ST):
                    for k in range(KD):
                        nc.tensor.matmul(
                            scores_ps[:, t, :],
                            xt_b[:, k, t * 128:(t + 1) * 128],
                            qp_sb[:, k, :],
                            start=(k == 0),
                            stop=(k == KD - 1),
                        )
                # exp(scores - SHIFT) -> unnormalized attention weights (fp32)
                nc.scalar.activation(
                    attn_b[:],
                    scores_ps[:],
                    mybir.ActivationFunctionType.Exp,
                    bias=negs[:],
                )

                # ctx[h, e] & Z: attnu.T @ [x | 1] (PE, fp32r)
                ctx_ps = psc.tile([H, 512], F32, tag="ctx")
                for t in range(ST):
                    nc.tensor.matmul(
                        ctx_ps[:, 0:D + 2],
                        attn_b[:, t, :],
                        xn32_b[:, t, :],
                        start=(t == 0),
                        stop=(t == ST - 1),
                    )
                # 1/Z from the ones column, then fold into ctx (fp16 out)
                nc.vector.reciprocal(recip[:, b, :], ctx_ps[:, D:D + 1])
                nc.vector.tensor_scalar_mul(
                    ctx_b[:],
                    ctx_ps[:, 0:D],
                    recip[:, b, :],
                )

                # ctxT[e, h] via PE transpose into the persistent gather
                for eh in range(KD):
                    ctp = pst.tile([128, H], F16, tag="tp")
                    nc.tensor.transpose(
                        ctp[:],
                        ctx_b[:, eh * 128:(eh + 1) * 128],
                        ident[:H, :H],
                    )
                    nc.vector.tensor_copy(ctxT_sb[:, eh, b, :], ctp[:])

            # ---- out[b, :] = sum_{h,eh} ctxT_(h,eh).T @ M_(h,eh) + bias
            out_ps = psf.tile([BL, D], F32, tag="fin")
            for h in range(H):
                for eh in range(KD):
                    k = h * KD + eh
                    nc.tensor.matmul(
                        out_ps[:],
                        ctxT_sb[:, eh, :, h],
                        m_sb[:, k, :],
                        start=(k == 0),
                        stop=False,
                    )
            nc.tensor.matmul(
                out_ps[:],
                ones_sb[:],
                bias_sb[:],
                start=False,
                stop=True,
            )
            nc.vector.tensor_copy(out_sb[:], out_ps[:])
            nc.sync.dma_start(out_d[:], out_sb[:])

    nc.compile()
    return nc


_NC_CACHE = []


def get_nc():
    if not _NC_CACHE:
        _NC_CACHE.append(build_program())
    return _NC_CACHE[0]


def make_packed(x, Wk, bk, Wv, bv, query, Wo, bo):
    """Pack the full problem (all B batches + transformed weights) into the
    single flat fp16 buffer the kernel reads."""
    x = np.ascontiguousarray(x, dtype=np.float32)
    xn1 = np.concatenate(
        [x, np.ones((x.shape[0], x.shape[1], 2), np.float32)], axis=2
    )
    wk = np.ascontiguousarray(Wk, dtype=np.float32)
    wv = np.ascontiguousarray(Wv, dtype=np.float32)
    wo = np.ascontiguousarray(Wo, dtype=np.float32)
    q = np.ascontiguousarray(query, dtype=np.float32)
    bvv = np.ascontiguousarray(bv, dtype=np.float32)
    bob = np.ascontiguousarray(bo, dtype=np.float32)

    # host weight-only transforms (all tiny vs the x-dependent work)
    # q_proj[e,h] = sum_d Wk[e, h*D+d] * query[h,d]; layout [128, KD, H]
    qp = np.einsum("ehd,hd->eh", wk.reshape(D, H, D), q).astype(np.float32)
    qp_host = np.ascontiguousarray(qp.reshape(KD, 128, H).transpose(1, 0, 2))
    # M[h] = Wv_h @ Wo_h; layout [128, KHD, D] with k = h*KD + eh, e = eh*128+p
    wv_h = np.ascontiguousarray(wv.reshape(D, H, D).transpose(1, 0, 2))  # [h,e,d]
    wo_h = wo.reshape(H, D, D)                                           # [h,d,n]
    m = np.matmul(wv_h, wo_h)                                            # [h,e,n]
    m_host = np.ascontiguousarray(m.reshape(KHD, 128, D).transpose(1, 0, 2))
    bias = (bvv @ wo + bob).astype(np.float32)

    # x pre-tiled so each SBUF partition's DMA read is one contiguous chunk:
    # [b, p, t, e] with s = t*128 + p
    xn_tiled = np.ascontiguousarray(
        xn1.reshape(B, ST, 128, D + 2).transpose(0, 2, 1, 3)
    )

    return np.concatenate(
        [xn_tiled.ravel(), qp_host.ravel(), m_host.ravel(), bias.ravel()]
    ).astype(np.float16)


def make_in_maps(x, Wk, bk, Wv, bv, query, Wo, bo):
    pk = make_packed(x, Wk, bk, Wv, bv, query, Wo, bo)
    return [{"pk": pk} for _ in range(NCORES)]


def kernel(x, Wk, bk, Wv, bv, query, Wo, bo):
    nc = get_nc()
    pk = make_packed(x, Wk, bk, Wv, bv, query, Wo, bo)
    res = run_bass_kernel_spmd(nc, [{"pk": pk}], core_ids=[0])
    return np.asarray(res.results[0]["out"])


# revision 17
# speedup vs baseline: 254.4262x; 1.0698x over previous
"""Trainium2 Bass kernel for CustomMultiHeadAttention (single-query pooled attention).

Reference computation (B=32, S=1024, D=256, H=8):
    keys   = (x @ Wk + bk).reshape(B,S,H,D)
    values = (x @ Wv + bv).reshape(B,S,H,D)
    scores = einsum('bshd,hd->bsh', keys, query)
    attn   = softmax(scores, axis=1)           # over S
    pooled = einsum('bsh,bshd->bhd', attn, values).reshape(B, H*D)
    out    = pooled @ Wo + bo

Algebraic restructure (exact in real arithmetic):
    q_proj[e,h] = sum_d Wk[e, h*D+d] * query[h,d]        # [256, 8]   (host)
    scores[b,s,h] = x[b,s,:] @ q_proj[:,h]  (+ const(h) from bk -> cancels in softmax)
    attnu = exp(scores - 64)                             # const shift; softmax invariant
    ctx[b,h,e]  = sum_s attnu[b,s,h] * x[b,s,e];  Z[b,h] = sum_s attnu[b,s,h]
    M[h] = Wv_h @ Wo_h                                   # [8, 256, 256] (host fuse)
    out[b,:] = sum_h (ctx[b,h,:]/Z[b,h]) @ M[h] + (bv @ Wo + bo)

This removes both [B*S,256]x[256,2048] projections AND the per-head value/output
GEMMs (fused on host into M = Wv_h @ Wo_h, a weight-only transform). On-chip work
is only what touches x: scores (x @ q_proj), the attnu.T @ [x|1] context matmul,
and the tiny [32,2048]x[2048,256] output GEMM. Z comes free as an extra all-ones
column appended to x in the ctx matmul.

The kernel is HBM/DMA-bound (it must stream all of x once per execution), so x
and the transformed weights ship as fp16 — halving DMA bytes. The softmax path
stays fp32: exp/Z/normalize run on fp32 (attn magnitudes span e^±50, far beyond
fp16 range), with x upconverted on-chip (DVE) for the fp32r ctx matmul.
Verified end-to-end relative error 1.4e-3 (fp32 baseline was 5.7e-4).

Execution shape: ONE core computes the FULL problem (all 32 batches), streaming
x batch-by-batch through multi-buffered SBUF tiles. Through the axon PJRT
tunnel the per-execute dispatch cost dominates actual kernel time, and that
dispatch pipeline parallelizes across independent per-device executables — so
for throughput the 8 cores run 8 independent full-problem instances
(replica-parallel serving) rather than co-operating on one instance.

All inputs are packed into ONE flat fp16 DRAM buffer, pre-tiled on host so each
SBUF partition's slice is one contiguous chunk (one ExternalInput + one
ExternalOutput: per-execute dispatch cost scales with buffer bindings).
"""

import sys

sys.path.insert(0, "/opt/trn_rl_repo")

import numpy as np

import concourse.bass as bass
import concourse.mybir as mybir
import concourse.tile as tile
from concourse import bacc
from concourse.bass_utils import run_bass_kernel_spmd
from concourse.masks import make_identity

F16 = mybir.dt.float16
F32 = mybir.dt.float32
F32R = mybir.dt.float32r

B, S, D, H = 32, 1024, 256, 8
NCORES = 8
BL = B                # every core computes the full problem
ST = S // 128         # s-tiles per batch = 8
KD = 2                # 256 = 2 k-tiles of 128 over the D (input dim) axis
KHD = (H * D) // 128  # 16 k-tiles over the H*D axis
SHIFT = 64.0          # constant score shift before exp (softmax-invariant)

# packed flat-buffer layout (fp16 elements, per core)
XN_SZ = S * (D + 2)            # one batch of x with 2 ones columns appended
QP_OFF = BL * XN_SZ            # q_proj [128, KD, H]
M_OFF = QP_OFF + D * H         # fused Wv@Wo [128, KHD, D]
BIAS_OFF = M_OFF + H * D * D   # bv @ Wo + bo [D]
NTOT = BIAS_OFF + D


def build_program():
    nc = bacc.Bacc(
        "TRN2", target_bir_lowering=False, debug=False, enable_partition_id=False
    )

    pk_d = nc.dram_tensor("pk", [NTOT], F16, kind="ExternalInput")
    out_d = nc.dram_tensor("out", [BL, D], F32, kind="ExternalOutput")

    with tile.TileContext(nc) as tc:
        with (
            tc.tile_pool(name="wts", bufs=1) as wts,
            tc.tile_pool(name="strm", bufs=4) as strm,
            tc.tile_pool(name="ps", bufs=2, space=bass.MemorySpace.PSUM) as ps,
            tc.tile_pool(name="psx", bufs=2, space=bass.MemorySpace.PSUM) as psx,
            tc.tile_pool(name="psc", bufs=2, space=bass.MemorySpace.PSUM) as psc,
            tc.tile_pool(name="pst", bufs=1, space=bass.MemorySpace.PSUM) as pst,
            tc.tile_pool(name="psf", bufs=1, space=bass.MemorySpace.PSUM) as psf,
        ):
            # ---- persistent SBUF ---------------------------------------
            m_sb = wts.tile([128, KHD, D], F16)           # fused Wv@Wo per (h,eh)
            qp_sb = wts.tile([128, KD, H], F16)           # q_proj [e, h]
            ctxT_sb = wts.tile([128, KD, BL, H], F16)     # [e%128, eh, b, h]
            recip = wts.tile([H, BL, 1], F32)             # 1/Z per (h, b)
            bias_sb = wts.tile([1, D], F16)               # bv @ Wo + bo
            ones_sb = wts.tile([1, BL], F16)
            ident = wts.tile([16, 16], F16)
            ident128 = wts.tile([128, 128], F16)
            negs = wts.tile([128, 1], F32)                # -SHIFT bias for exp
            out_sb = wts.tile([BL, D], F32)

            nc.sync.dma_start(
                qp_sb[:],
                pk_d[QP_OFF:QP_OFF + D * H]
                .rearrange("(p k h) -> p k h", k=KD, h=H),
            )
            nc.sync.dma_start(
                m_sb[:],
                pk_d[M_OFF:M_OFF + H * D * D]
                .rearrange("(p k n) -> p k n", k=KHD, n=D),
            )
            nc.sync.dma_start(
                bias_sb[:],
                pk_d[BIAS_OFF:BIAS_OFF + D].rearrange("d -> () d"),
            )
            make_identity(nc, ident[:])
            make_identity(nc, ident128[:])
            nc.vector.memset(negs[:], -SHIFT)
            nc.vector.memset(ones_sb[:], 1.0)

            # ---- stream one batch at a time ---------------------------
            for b in range(BL):
                xn_b = strm.tile([128, ST, D + 2], F16, tag="xn")
                xn32_b = strm.tile([128, ST, D + 2], F32R, tag="xn32")
                xt_b = strm.tile([128, KD, S], F16, tag="xt")
                attn_b = strm.tile([128, ST, H], F32R, tag="attn")
                ctx_b = strm.tile([H, D], F16, tag="ctx")

                nc.sync.dma_start(
                    xn_b[:].rearrange("p t e -> p (t e)"),
                    pk_d[b * XN_SZ:(b + 1) * XN_SZ]
                    .rearrange("(p n) -> p n", n=XN_SZ // 128),
                )
                # fp32 copy of x for the fp32 softmax/ctx path
                nc.vector.tensor_copy(xn32_b[:], xn_b[:])

                # transpose x: xt[e, s]; 4 transposes share one PSUM bank
                for tp2 in range(ST // 2):       # pairs of s-tiles
                    xtp = psx.tile([128, 2, 2, 128], F16, tag="xtp")
                    for toff in range(2):
                        t = tp2 * 2 + toff
                        for eh in range(KD):
                            nc.tensor.transpose(
                                xtp[:, toff, eh, :],
                                xn_b[:, t, eh * 128:(eh + 1) * 128],
                                ident128[:],
                            )
                    nc.vector.tensor_copy(
                        xt_b[:, :, tp2 * 256:(tp2 + 1) * 256]
                        .rearrange("p k (o s) -> p k o s", o=2),
                        xtp[:].rearrange("p o k s -> p k o s"),
                    )

                # scores[s, h] = xt_tile.T @ q_proj (fp16 operands, fp32 accum)
                scores_ps = ps.tile([128, ST, H], F32, tag="scores")
                for t in range(ST):
                    for k in range(KD):
                        nc.tensor.matmul(
                            scores_ps[:, t, :],
                            xt_b[:, k, t * 128:(t + 1) * 128],
                            qp_sb[:, k, :],
                            start=(k == 0),
                            stop=(k == KD - 1),
                        )
                # exp(scores - SHIFT) -> unnormalized attention weights (fp32)
                nc.scalar.activation(
                    attn_b[:],
                    scores_ps[:],
                    mybir.ActivationFunctionType.Exp,
                    bias=negs[:],
                )

                # ctx[h, e] & Z: attnu.T @ [x | 1] (PE, fp32r)
                ctx_ps = psc.tile([H, 512], F32, tag="ctx")
                for t in range(ST):
                    nc.tensor.matmul(
                        ctx_ps[:, 0:D + 2],
                        attn_b[:, t, :],
                        xn32_b[:, t, :],
                        start=(t == 0),
                        stop=(t == ST - 1),
                    )
                # 1/Z from the ones column, then fold into ctx (fp16 out)
                nc.vector.reciprocal(recip[:, b, :], ctx_ps[:, D:D + 1])
                nc.vector.tensor_scalar_mul(
                    ctx_b[:],
                    ctx_ps[:, 0:D],
                    recip[:, b, :],
                )

                # ctxT[e, h] via PE transpose into the persistent gather
                for eh in range(KD):
                    ctp = pst.tile([128, H], F16, tag="tp")
                    nc.tensor.transpose(
                        ctp[:],
                        ctx_b[:, eh * 128:(eh + 1) * 128],
                        ident[:H, :H],
                    )
                    nc.vector.tensor_copy(ctxT_sb[:, eh, b, :], ctp[:])

            # ---- out[b, :] = sum_{h,eh} ctxT_(h,eh).T @ M_(h,eh) + bias
            out_ps = psf.tile([BL, D], F32, tag="fin")
            for h in range(H):
                for eh in range(KD):
                    k = h * KD + eh
                    nc.tensor.matmul(
                        out_ps[:],
                        ctxT_sb[:, eh, :, h],
                        m_sb[:, k, :],
                        start=(k == 0),
                        stop=False,
                    )
            nc.tensor.matmul(
                out_ps[:],
                ones_sb[:],
                bias_sb[:],
                start=False,
                stop=True,
            )
            nc.vector.tensor_copy(out_sb[:], out_ps[:])
            nc.sync.dma_start(out_d[:], out_sb[:])

    nc.compile()
    return nc


_NC_CACHE = []


def get_nc():
    if not _NC_CACHE:
        _NC_CACHE.append(build_program())
    return _NC_CACHE[0]


def make_packed(x, Wk, bk, Wv, bv, query, Wo, bo):
    """Pack the full problem (all B batches + transformed weights) into the
    single flat fp16 buffer the kernel reads."""
    x = np.ascontiguousarray(x, dtype=np.float32)
    xn1 = np.concatenate(
        [x, np.ones((x.shape[0], x.shape[1], 2), np.float32)], axis=2
    )
    wk = np.ascontiguousarray(Wk, dtype=np.float32)
    wv = np.ascontiguousarray(Wv, dtype=np.float32)
    wo = np.ascontiguousarray(Wo, dtype=np.float32)
    q = np.ascontiguousarray(query, dtype=np.float32)
    bvv = np.ascontiguousarray(bv, dtype=np.float32)
    bob = np.ascontiguousarray(bo, dtype=np.float32)

    # host weight-only transforms (all tiny vs the x-dependent work)
    # q_proj[e,h] = sum_d Wk[e, h*D+d] * query[h,d]; layout [128, KD, H]
    qp = np.einsum("ehd,hd->eh", wk.reshape(D, H, D), q).astype(np.float32)
    qp_host = np.ascontiguousarray(qp.reshape(KD, 128, H).transpose(1, 0, 2))
    # M[h] = Wv_h @ Wo_h; layout [128, KHD, D] with k = h*KD + eh, e = eh*128+p
    wv_h = np.ascontiguousarray(wv.reshape(D, H, D).transpose(1, 0, 2))  # [h,e,d]
    wo_h = wo.reshape(H, D, D)                                           # [h,d,n]
    m = np.matmul(wv_h, wo_h)                                            # [h,e,n]
    m_host = np.ascontiguousarray(m.reshape(KHD, 128, D).transpose(1, 0, 2))
    bias = (bvv @ wo + bob).astype(np.float32)

    # x pre-tiled so each SBUF partition's DMA read is one contiguous chunk:
    # [b, p, t, e] with s = t*128 + p
    xn_tiled = np.ascontiguousarray(
        xn1.reshape(B, ST, 128, D + 2).transpose(0, 2, 1, 3)
    )

    return np.concatenate(
        [xn_tiled.ravel(), qp_host.ravel(), m_host.ravel(), bias.ravel()]
    ).astype(np.float16)


def make_in_maps(x, Wk, bk, Wv, bv, query, Wo, bo):
    pk = make_packed(x, Wk, bk, Wv, bv, query, Wo, bo)
    return [{"pk": pk} for _ in range(NCORES)]


def kernel(x, Wk, bk, Wv, bv, query, Wo, bo):
    nc = get_nc()
    pk = make_packed(x, Wk, bk, Wv, bv, query, Wo, bo)
    res = run_bass_kernel_spmd(nc, [{"pk": pk}], core_ids=[0])
    return np.asarray(res.results[0]["out"])
